# revision 17
# baseline (speedup 1.0000x reference)
"""Focal Gaussian loss (EDT heatmap + focal MSE) on 8 Trainium2 cores.

Data-parallel over batch: each core processes B/8 = 2 images,
producing per-image partial sums (sum of focal_factor, sum of
focal*mse). The host combines the partials from all cores and applies
the global normalization:

    out = SCALE * mean(focal*mse) / (mean(focal) + 0.01)

Row EDT: count-since-last-fg recurrence via DVE scans (forward +
reverse), with a sentinel column between row-chunks so one scan
instruction covers a whole image. Column pass: min-conv with the
parabola t^2 over a +-T window in bf16 (d2 integers <= 256 are exact in
bf16; any pixel the window or rounding affects has heatmap <
exp(-T^2/8) = e^-8, measured scalar impact ~1e-6 relative). DVE does
bf16 2x mins in a 3-chain tree (short serial depth), ACT the parabola
adds, PE the row<->column transposes, GpSimd the f32 focal prep. The
two images pipeline against each other across engines.
"""

import numpy as np

B, H, W = 16, 512, 512
N_CORES = 8
IPC = B // N_CORES  # images per core
T = 8               # column min-conv window radius
W1 = W + 1          # row width + sentinel column
SENT = 64.0         # sentinel invt value: forces count >= 64 at row starts
BIG = 1.0e6
BIG2 = 1.0e12
SCALE = 2.0
EPS = 0.01
P = 128
RB = H // P  # row blocks
CB = W // P  # col blocks
WPAD = 512 + 2 * T

_CACHE = {}


def build_program():
    import concourse.bacc as bacc
    import concourse.mybir as mybir
    import concourse.tile as tile

    f32 = mybir.dt.float32
    bf16 = mybir.dt.bfloat16
    Alu = mybir.AluOpType
    Act = mybir.ActivationFunctionType

    nc = bacc.Bacc(
        "TRN2", target_bir_lowering=False, debug=False, num_devices=N_CORES
    )

    inp_d = nc.dram_tensor("inputs", [IPC, H, W], f32, kind="ExternalInput").ap()
    tgt_d = nc.dram_tensor("targets", [IPC, H, W], f32, kind="ExternalInput").ap()
    identb_d = nc.dram_tensor("identb", [P, P], bf16, kind="ExternalInput").ap()
    tsq_d = nc.dram_tensor("tsq", [P, T + 1], f32, kind="ExternalInput").ap()
    part_d = nc.dram_tensor("partials", [P, 2 * IPC], f32, kind="ExternalOutput").ap()

    with tile.TileContext(nc) as tc:
        with (
            tc.tile_pool(name="const", bufs=1) as cpool,
            tc.tile_pool(name="io", bufs=1) as iopool,
            tc.tile_pool(name="work", bufs=1) as wpool,
            tc.tile_pool(name="f2Tp", bufs=2) as fpool,
            tc.tile_pool(name="pmp", bufs=2) as pmpool,
            tc.tile_pool(name="tmp", bufs=3) as tpool,
            tc.tile_pool(name="chain", bufs=3) as hpool,
            tc.tile_pool(name="psum", bufs=4, space="PSUM") as ppool,
        ):
            identb = cpool.tile([P, P], bf16)
            nc.sync.dma_start(identb[:], identb_d[:])
            tsq = cpool.tile([P, T + 1], f32)
            nc.sync.dma_start(tsq[:], tsq_d[:])
            ones = cpool.tile([P, RB * W1], f32)
            nc.vector.memset(ones[:], 1.0)
            bias015 = cpool.tile([P, 1], f32)
            nc.vector.memset(bias015[:], 0.15)
            partials = cpool.tile([P, 2 * IPC], f32)

            tgt = iopool.tile([P, IPC, RB, W], f32, tag="tgt")
            inp = iopool.tile([P, IPC, RB, W], f32, tag="inp")
            tgt_r = tgt_d.rearrange("i (a p) w -> p i a w", p=P)
            inp_r = inp_d.rearrange("i (a p) w -> p i a w", p=P)
            for i in range(IPC):
                nc.sync.dma_start(tgt[:, i], tgt_r[:, i])
            for i in range(IPC):
                nc.sync.dma_start(inp[:, i], inp_r[:, i])

            invt = wpool.tile([P, IPC, RB, W1], f32, tag="invt")
            dl = wpool.tile([P, IPC, RB, W1], f32, tag="dl")
            dr = wpool.tile([P, IPC, RB, W1], f32, tag="dr")
            pred = wpool.tile([P, IPC, RB, W], f32, tag="pred")
            sql = wpool.tile([P, IPC, RB, W], bf16, tag="sql")
            sqr = wpool.tile([P, IPC, RB, W], bf16, tag="sqr")
            acc = wpool.tile([P, IPC, CB, 512], bf16, tag="acc")
            heat = inp  # reuse after sigmoid
            wt = dl     # reuse after squares
            q_ = invt   # reuse after scans
            q2 = dr     # reuse after squares
            alpha = tgt  # computed in place over tgt (its last consumer)

            nc.vector.memset(invt[:, :, :, W:W1], SENT)

            # chain assignment: radii grouped {1,4,7},{2,5,8},{3,6}
            CH = [[1, 4, 7], [2, 5, 8], [3, 6]]

            for i in range(IPC):
                # --- row EDT ---
                if i == 0:
                    nc.vector.tensor_scalar(
                        invt[:, i, :, 0:W], tgt[:, i], -1.0, 1.0,
                        Alu.mult, Alu.add,
                    )
                else:
                    nc.gpsimd.tensor_scalar(
                        invt[:, i, :, 0:W], tgt[:, i], -1.0, 1.0,
                        Alu.mult, Alu.add,
                    )
                iflat = invt[:, i].rearrange("p a w -> p (a w)")
                lflat = dl[:, i].rearrange("p a w -> p (a w)")
                rflat = dr[:, i].rearrange("p a w -> p (a w)")
                nc.vector.tensor_tensor_scan(
                    lflat, ones[:], iflat, BIG, Alu.add, Alu.mult
                )
                nc.vector.tensor_tensor_scan(
                    rflat[:, ::-1], ones[:], iflat[:, ::-1], BIG,
                    Alu.add, Alu.mult,
                )
                nc.scalar.activation(
                    pred[:, i], inp[:, i], Act.Sigmoid
                )
                nc.scalar.square(sql[:, i], dl[:, i, :, 0:W])
                nc.scalar.square(sqr[:, i], dr[:, i, :, 0:W])
                f2r_i = sql[:, i]
                nc.vector.tensor_tensor(
                    f2r_i, sql[:, i], sqr[:, i], op=Alu.min
                )

                # --- transpose to column-major, +-T row padding ---
                f2T = fpool.tile([P, CB, WPAD], bf16, tag="f2T")
                nc.vector.memset(f2T[:, :, 0:T], BIG2)
                nc.vector.memset(f2T[:, :, T + 512 : WPAD], BIG2)
                for cb in range(CB):
                    ps = ppool.tile([P, 512], bf16, tag="psT")
                    for rb in range(RB):
                        nc.tensor.transpose(
                            ps[:, rb * P : (rb + 1) * P],
                            f2r_i[:, rb, cb * P : (cb + 1) * P],
                            identb[:],
                        )
                    nc.scalar.copy(f2T[:, cb, T : T + 512], ps[:])

                # --- column min-conv, 3-chain tree ---
                def sl(off):
                    return f2T[:, :, T + off : T + off + 512]

                chains = []
                for group in CH:
                    cacc = None
                    for r in group:
                        pm = pmpool.tile([P, CB, 512], bf16, tag="pm")
                        nc.vector.tensor_tensor(
                            pm[:], sl(-r), sl(r), op=Alu.min
                        )
                        tm = tpool.tile([P, CB, 512], bf16, tag="tm")
                        nc.scalar.activation(
                            tm[:], pm[:], Act.Identity, bias=tsq[:, r : r + 1]
                        )
                        if cacc is None:
                            cacc = hpool.tile([P, CB, 512], bf16, tag="ch")
                            nc.vector.tensor_copy(cacc[:], tm[:])
                        else:
                            nc.vector.tensor_tensor(
                                cacc[:], cacc[:], tm[:], op=Alu.min
                            )
                    chains.append(cacc)
                # merge: min(c0, c1), min(c2, center), then final
                m1 = chains[0]
                nc.vector.tensor_tensor(
                    m1[:], chains[0][:], chains[1][:], op=Alu.min
                )
                m2 = chains[2]
                nc.vector.tensor_tensor(
                    m2[:], chains[2][:], sl(0), op=Alu.min
                )
                nc.vector.tensor_tensor(
                    acc[:, i], m1[:], m2[:], op=Alu.min
                )

                # --- focal prep on GpSimd (overlaps DVE/ACT work) ---
                nc.gpsimd.tensor_scalar(
                    wt[:, i, :, 0:W], tgt[:, i], -2.0, 1.0, Alu.mult, Alu.add
                )
                nc.gpsimd.tensor_tensor(
                    q_[:, i, :, 0:W], pred[:, i], wt[:, i, :, 0:W],
                    op=Alu.mult,
                )
                nc.gpsimd.tensor_tensor(
                    q_[:, i, :, 0:W], q_[:, i, :, 0:W], tgt[:, i], op=Alu.add
                )
                # alpha_t = 0.7*pos + 0.15, in place over tgt (last use)
                nc.scalar.activation(
                    alpha[:, i], tgt[:, i], Act.Identity,
                    bias=bias015[:], scale=0.7,
                )
                nc.scalar.square(q2[:, i, :, 0:W], q_[:, i, :, 0:W])
                nc.vector.scalar_tensor_tensor(
                    q2[:, i, :, 0:W], alpha[:, i], 1.0, q2[:, i, :, 0:W],
                    Alu.mult, Alu.mult,
                    accum_out=partials[:, 2 * i : 2 * i + 1],
                )

                # --- transpose back + heat = exp(-d2/8), then mse tail ---
                for rb in range(RB):
                    ph = ppool.tile([P, 512], bf16, tag="psH")
                    for cb in range(CB):
                        nc.tensor.transpose(
                            ph[:, cb * P : (cb + 1) * P],
                            acc[:, i, cb, rb * P : (rb + 1) * P],
                            identb[:],
                        )
                    nc.scalar.activation(
                        heat[:, i, rb], ph[:], Act.Exp, scale=-0.125
                    )
                    nc.gpsimd.tensor_tensor(
                        pred[:, i, rb], pred[:, i, rb], heat[:, i, rb],
                        op=Alu.subtract,
                    )
                nc.vector.tensor_mul(pred[:, i], pred[:, i], pred[:, i])
                nc.vector.scalar_tensor_tensor(
                    pred[:, i], q2[:, i, :, 0:W], 1.0, pred[:, i],
                    Alu.mult, Alu.mult,
                    accum_out=partials[:, 2 * i + 1 : 2 * i + 2],
                )

            nc.sync.dma_start(part_d[:], partials[:])

    nc.compile()
    return nc


def host_constants():
    import ml_dtypes

    identb = np.eye(P, dtype=ml_dtypes.bfloat16)
    tsq = np.broadcast_to(
        (np.arange(T + 1, dtype=np.float32) ** 2), (P, T + 1)
    ).copy()
    return identb, tsq


def make_in_maps(inputs, targets):
    identb, tsq = host_constants()
    in_maps = []
    for c in range(N_CORES):
        sl_ = slice(c * IPC, (c + 1) * IPC)
        in_maps.append(
            {
                "inputs": np.ascontiguousarray(inputs[sl_, 0]),
                "targets": np.ascontiguousarray(targets[sl_, 0]),
                "identb": identb,
                "tsq": tsq,
            }
        )
    return in_maps


def combine_partials(partial_list):
    """partial_list: one [128, 2*IPC] array per core -> final scalar."""
    sf = 0.0
    sl_ = 0.0
    for parts in partial_list:
        p64 = parts.astype(np.float64)
        sf += p64[:, 0::2].sum()
        sl_ += p64[:, 1::2].sum()
    n = float(B * H * W)
    out = SCALE * (sl_ / n) / (sf / n + EPS)
    return np.float32(out)


def kernel(inputs, targets):
    from concourse.bass_utils import run_bass_kernel_spmd

    if "nc" not in _CACHE:
        _CACHE["nc"] = build_program()
    nc = _CACHE["nc"]

    in_maps = make_in_maps(inputs, targets)
    res = run_bass_kernel_spmd(nc, in_maps, list(range(N_CORES)))
    return combine_partials([r["partials"] for r in res.results])
